# revision 30
# baseline (speedup 1.0000x reference)
"""Trainium2 Bass kernel for perturbed top-k patch extraction (topk_masking).

Contract: kernel(x_high, scores_2d) -> (patches, entr) matching the jax
reference. Batch (8) is sharded 1:1 across 8 NeuronCores; each core runs the
full per-batch pipeline on device:
  1. entropy + min-max normalization of scores [1,256]
  2. perturbed top-10 over 500 noise samples -> per-slot index distribution
     ind[k,d] via matmul-with-triangular-matrix ranking (no per-sample sort)
  3. patches einsum recast as parity-class block matmuls: x_pad decomposes
     into 32x32 blocks aligned with the input's 32-grid (the pad offset is
     exactly one block). Output quadrant (a,b) of patch k only touches blocks
     with (p,q) == (a,b) mod 2, so blocks group into 4 parity classes of 256;
     out[(a,b,k), pix] = sum_pq coeff[pq,(a,b,k)] * B32[pq, pix] with
     coeff[pq,(a,b,k)] = ind[k,(p-a)/2,(q-b)/2], built on device by matmul
     with constant 0/1 selection matrices.

The host only reshards: batch slicing, a layout permutation of x_high into
block order, and the final stack/mean. All arithmetic runs on device.
"""

import sys

if "/opt/trn_rl_repo" not in sys.path:
    sys.path.insert(0, "/opt/trn_rl_repo")

from contextlib import ExitStack

import ml_dtypes
import numpy as np

import concourse.bacc as bacc
import concourse.bass as bass
import concourse.mybir as mybir
import concourse.tile as tile
from concourse.bass_utils import run_bass_kernel_spmd

F32 = mybir.dt.float32
BF16 = mybir.dt.bfloat16
AX = mybir.AxisListType
OP = mybir.AluOpType
ACT = mybir.ActivationFunctionType

K = 10
NS = 500
SIGMA = 0.05
D = 256
N_CORES = 8
NCH = 4          # noise-row chunks of 125
ROWS = 125

# parity classes: for class (al, bl), p in [1,32] with p%2==al, q likewise.
_PLIST = {al: [p for p in range(1, 33) if p % 2 == al] for al in (0, 1)}

_CACHE = {}
_RUN_KWARGS = {}


def _build_nc():
    nc = bacc.Bacc(None, target_bir_lowering=False)
    xblk_ext = nc.dram_tensor("xblk", [4, 2, 3, 128, 1024], BF16, kind="ExternalInput")
    sc_ext = nc.dram_tensor("scores", [1, 256], F32, kind="ExternalInput")
    nz_ext = nc.dram_tensor("noise", [NS, D], F32, kind="ExternalInput")
    tri_ext = nc.dram_tensor("tri", [128, 128], BF16, kind="ExternalInput")
    id_ext = nc.dram_tensor("ident", [128, 128], F32, kind="ExternalInput")
    ssel_ext = nc.dram_tensor("ssel", [128, 8192], BF16, kind="ExternalInput")
    pout_ext = nc.dram_tensor("pout", [40, 12288], F32, kind="ExternalOutput")
    entr_ext = nc.dram_tensor("entr", [1, 1], F32, kind="ExternalOutput")

    with tile.TileContext(nc) as tc, ExitStack() as ctx:
        const = ctx.enter_context(tc.tile_pool(name="const", bufs=1))
        resid = ctx.enter_context(tc.tile_pool(name="resid", bufs=1))
        work = ctx.enter_context(tc.tile_pool(name="work", bufs=2))
        pers = ctx.enter_context(tc.tile_pool(name="pers", bufs=1))
        ps = ctx.enter_context(tc.tile_pool(name="ps", bufs=1, space=bass.MemorySpace.PSUM))

        # ---- constants ----
        sc = const.tile([1, D], F32)
        nc.scalar.dma_start(sc[:], sc_ext[:])
        tri = const.tile([128, 128], BF16)
        nc.scalar.dma_start(tri[:], tri_ext[:])
        ident = const.tile([128, 128], F32)
        nc.scalar.dma_start(ident[:], id_ext[:])
        ssel = const.tile([128, 8192], BF16)
        nc.sync.dma_start(ssel[:], ssel_ext[:])
        ones128 = const.tile([128, 128], BF16)
        nc.vector.memset(ones128[:], 1.0)
        onescol = const.tile([1, 128], F32)
        nc.vector.memset(onescol[:], 1.0)

        # ---- resident 32x32 blocks, one [128, 1024] tile per (cls, ch, c) ----
        xb = [
            [
                [
                    resid.tile([128, 1024], BF16, tag=f"xb{cls}{ch}{c}", name=f"xb{cls}{ch}{c}")
                    for c in range(3)
                ]
                for ch in range(2)
            ]
            for cls in range(4)
        ]
        for cls in range(4):
            for ch in range(2):
                for c in range(3):
                    nc.sync.dma_start(xb[cls][ch][c][:], xblk_ext[cls, ch, c])

        # ---- stage A: entropy + min-max norm of scores ----
        mx = work.tile([1, 1], F32, tag="mx")
        nc.vector.reduce_max(mx[:], sc[:], axis=AX.X)
        mn = work.tile([1, 1], F32, tag="mn")
        nc.vector.tensor_reduce(mn[:], sc[:], axis=AX.X, op=OP.min)
        rng = work.tile([1, 1], F32, tag="rng")
        nc.vector.tensor_tensor(rng[:], mx[:], mn[:], op=OP.subtract)
        nc.vector.tensor_scalar_add(rng[:], rng[:], 1e-5)
        rinv = work.tile([1, 1], F32, tag="rinv")
        nc.vector.reciprocal(rinv[:], rng[:])
        nrm = pers.tile([1, D], F32, tag="nrm")
        nc.vector.tensor_scalar(
            nrm[:], sc[:], mn[:], rinv[:], op0=OP.subtract, op1=OP.mult
        )
        # entropy (independent of the norm chain; runs whenever):
        # entr = logZ - sum(e*s)/Z  with s = sc - mx, e = exp(s)
        e_s = work.tile([1, D], F32, tag="e_s")
        nc.vector.tensor_scalar(e_s[:], sc[:], mx[:], None, op0=OP.subtract)
        e_t = work.tile([1, D], F32, tag="e_t")
        zacc = work.tile([1, 1], F32, tag="zacc")
        nc.scalar.activation(e_t[:], e_s[:], ACT.Exp, accum_out=zacc[:])
        es_t = work.tile([1, D], F32, tag="es_t")
        nc.vector.tensor_tensor(es_t[:], e_t[:], e_s[:], op=OP.mult)
        es = work.tile([1, 1], F32, tag="es")
        nc.vector.reduce_sum(es[:], es_t[:], axis=AX.X)
        rz = work.tile([1, 1], F32, tag="rz")
        nc.vector.reciprocal(rz[:], zacc[:])
        lz = work.tile([1, 1], F32, tag="lz")
        nc.scalar.activation(lz[:], zacc[:], ACT.Ln)
        ent = work.tile([1, 1], F32, tag="ent")
        nc.vector.tensor_tensor(ent[:], es[:], rz[:], op=OP.mult)
        nc.vector.tensor_tensor(ent[:], lz[:], ent[:], op=OP.subtract)
        nc.scalar.dma_start(entr_ext[:], ent[:])

        # broadcast nrm to 128 partitions via K=1 matmul
        normb = ps.tile([128, D], F32)
        nc.tensor.matmul(normb[:], onescol[:], nrm[:], start=True, stop=True)

        # ---- stage B/C: perturbed top-10 membership, 4 chunks interleaved ----
        # Per iteration: eqn = (P==mval)*-2^100 (dual-op tensor_scalar), then
        # P += eqn (split DVE/Pool) and a fresh row-max. P_final - P_0 equals
        # -2^100 * membership, so no separate accumulator is needed; the
        # -2^-100 rescale rides the PSUM->SBUF transpose copies.
        mt = [pers.tile([128, NS], BF16, tag=f"mt{dc}", name=f"mt{dc}") for dc in range(2)]
        big = float(2.0**100)
        p_t, p_0, mvals = [], [], []
        for nch in range(NCH):
            nzt = work.tile([ROWS, D], F32, tag=f"nz{nch}", name=f"nz{nch}")
            nc.scalar.dma_start(nzt[:], nz_ext[ROWS * nch : ROWS * (nch + 1), :])
            p0 = pers.tile([ROWS, D], F32, tag=f"p0_{nch}", name=f"p0_{nch}")
            nc.vector.scalar_tensor_tensor(
                p0[:], nzt[:], SIGMA, normb[0:ROWS, :], op0=OP.mult, op1=OP.add
            )
            pt = pers.tile([ROWS, D], F32, tag=f"P{nch}", name=f"P{nch}")
            nc.vector.tensor_copy(pt[:], p0[:])
            mval = pers.tile([ROWS, 1], F32, tag=f"mval{nch}", name=f"mval{nch}")
            nc.vector.reduce_max(mval[:], pt[:], axis=AX.X)
            p_t.append(pt)
            p_0.append(p0)
            mvals.append(mval)
        for it in range(K):
            for nch in range(NCH):
                # eqn = (P == max) * -2^100; {0, -2^100} exact even in bf16
                eqn = work.tile([ROWS, D], BF16, tag=f"eqn{nch}", name=f"eqn{nch}")
                nc.vector.tensor_scalar(
                    eqn[:], p_t[nch][:], mvals[nch][:], -big,
                    op0=OP.is_equal, op1=OP.mult,
                )
                # P accumulates every eqn; P_final - P_0 = -2^100 * M, so no
                # separate membership accumulator is needed.
                nc.gpsimd.tensor_tensor(p_t[nch][:], p_t[nch][:], eqn[:], op=OP.add)
                if it < K - 1:
                    nc.vector.reduce_max(mvals[nch][:], p_t[nch][:], axis=AX.X)
        msc = []
        for nch in range(NCH):
            dm = work.tile([ROWS, D], F32, tag=f"dm{nch}", name=f"dm{nch}")
            nc.vector.tensor_tensor(dm[:], p_t[nch][:], p_0[nch][:], op=OP.subtract)
            msc.append(dm)
        for nch in range(NCH):
            for dc in range(2):
                tp = ps.tile([128, ROWS], F32, tag="tpct", bufs=6, padded_shape=[128, NS])
                nc.tensor.transpose(
                    tp[:], msc[nch][:, dc * 128 : (dc + 1) * 128], ident[0:ROWS, 0:ROWS]
                )
                # cast + exact rescale back to {0,1}
                nc.scalar.mul(mt[dc][:, ROWS * nch : ROWS * (nch + 1)], tp[:], -(2.0**-100))

        # ---- stage D: rank counts Ct[d,n] = #selected d' < d, via L matmul ----
        ct = [ps.tile([128, NS], F32, tag="tpct", bufs=6, name=f"ct{dc}") for dc in range(2)]
        nc.tensor.matmul(ct[0][:], tri[:], mt[0][:], start=True, stop=True)
        nc.tensor.matmul(ct[1][:], ones128[:], mt[0][:], start=True, stop=False)
        nc.tensor.matmul(ct[1][:], tri[:], mt[1][:], start=False, stop=True)

        # R = M * (C+1) in {0..10}; indT[d,k] = sum_n [R == k+1] (scaled later)
        indt = [pers.tile([128, K], F32, tag=f"indt{dc}", name=f"indt{dc}") for dc in range(2)]
        indtb = [pers.tile([128, K], BF16, tag=f"indtb{dc}", name=f"indtb{dc}") for dc in range(2)]
        rt = [pers.tile([128, NS], BF16, tag=f"rt{dc}", name=f"rt{dc}") for dc in range(2)]
        for dc in range(2):
            # R = M * (C+1), small ints, exact in bf16
            nc.vector.tensor_scalar_add(rt[dc][:], ct[dc][:], 1.0)
            nc.vector.tensor_tensor(rt[dc][:], rt[dc][:], mt[dc][:], op=OP.mult)
        for dc in range(2):
            for k in range(K):
                eqk = work.tile([128, NS], BF16, tag=f"eqk{dc}", name=f"eqk{dc}")
                nc.vector.tensor_scalar(
                    eqk[:], rt[dc][:], float(k + 1), None, op0=OP.is_equal,
                    op1=OP.add, accum_out=indt[dc][:, k : k + 1],
                )
            nc.vector.tensor_copy(indtb[dc][:], indt[dc][:])

        # ---- stage F: coeff[pq,(ab,k)] = ind-count[d(pq,ab), k] via matmul
        # with constant 0/1 selection matrices; 1/NS rides the out copies.
        coeff = [
            [pers.tile([128, 40], BF16, tag=f"coef{cls}{ch}", name=f"coef{cls}{ch}") for ch in range(2)]
            for cls in range(4)
        ]
        for cls in range(4):
            for ch in range(2):
                for ab in range(4):
                    pc = ps.tile([128, K], F32, tag="pc")
                    for dc in range(2):
                        off = (((cls * 2 + ch) * 4 + ab) * 2 + dc) * 128
                        nc.tensor.matmul(
                            pc[:],
                            ssel[:, off : off + 128],
                            indtb[dc][:],
                            start=(dc == 0),
                            stop=(dc == 1),
                        )
                    nc.scalar.copy(coeff[cls][ch][:, ab * K : (ab + 1) * K], pc[:])

        # ---- main: out[(a,b,k), pix] = sum_pq coeff[pq,abk] * B32[pq,pix] ----
        # out_sb[(as,bs,k), (cls, c, n2, hl, ww)]; host unpermutes to patches.
        out_sb = pers.tile([40, 12288], F32, tag="out_sb")
        nco = 0
        for cls in range(4):
            for c in range(3):
                for n2 in range(2):
                    po = ps.tile([40, 512], F32, tag="tpct", bufs=6, name="po")
                    for ch in range(2):
                        nc.tensor.matmul(
                            po[:],
                            coeff[cls][ch][:],
                            xb[cls][ch][c][:, n2 * 512 : (n2 + 1) * 512],
                            start=(ch == 0),
                            stop=(ch == 1),
                        )
                    off = (cls * 3 + c) * 1024 + n2 * 512
                    if nco % 4 == 3:
                        nc.scalar.mul(out_sb[:, off : off + 512], po[:], 1.0 / NS)
                    else:
                        nc.vector.tensor_scalar_mul(out_sb[:, off : off + 512], po[:], 1.0 / NS)
                    nco += 1
            nc.scalar.dma_start(
                pout_ext[:, cls * 3072 : (cls + 1) * 3072],
                out_sb[:, cls * 3072 : (cls + 1) * 3072],
            )

    nc.finalize()
    return nc


def _host_consts():
    tri = np.zeros((128, 128), np.float32)
    for dp in range(128):
        tri[dp, dp + 1 :] = 1.0
    tri = tri.astype(ml_dtypes.bfloat16)
    ident = np.eye(128, dtype=np.float32)
    s_host = np.zeros((128, 4, 2, 4, 2, 128), np.float32)
    for cls in range(4):
        al, bl = cls // 2, cls % 2
        plist, qlist = _PLIST[al], _PLIST[bl]
        for ch in range(2):
            for pi in range(8):
                p = plist[ch * 8 + pi]
                for qi in range(16):
                    q = qlist[qi]
                    pq = pi * 16 + qi
                    for a_s in range(2):
                        for b_s in range(2):
                            a, b = 2 * a_s + al, 2 * b_s + bl
                            i2, j2 = (p - a) // 2, (q - b) // 2
                            if 0 <= i2 < 16 and 0 <= j2 < 16:
                                d = i2 * 16 + j2
                                dc, dd = divmod(d, 128)
                                s_host[dd, cls, ch, a_s * 2 + b_s, dc, pq] = 1.0
    ssel = s_host.reshape(128, 8192).astype(ml_dtypes.bfloat16)
    return tri, ident, ssel


def _blockize(x_b):
    """x_b [3,1024,1024] -> [4 cls, 2 ch, 3 c, 128 pq, 1024 pix] block layout."""
    blk = x_b.reshape(3, 32, 32, 32, 32).transpose(1, 3, 0, 2, 4)  # [P, Q, c, h, w]
    out = np.empty((4, 2, 3, 128, 1024), ml_dtypes.bfloat16)
    for cls in range(4):
        al, bl = cls // 2, cls % 2
        sel_p = np.array(_PLIST[al]) - 1
        sel_q = np.array(_PLIST[bl]) - 1
        sub = blk[np.ix_(sel_p, sel_q)]  # [16, 16, 3, 32, 32]
        for ch in range(2):
            part = sub[ch * 8 : (ch + 1) * 8]  # [8, 16, 3, 32, 32]
            out[cls, ch] = part.transpose(2, 0, 1, 3, 4).reshape(3, 128, 1024)
    return out


def _unpack_out(pout):
    """pout [40, 12288] -> patches [10, 3, 128, 128].

    pout[(as,bs,k), (cls, c, n2, hl, ww)] with cls=(al,bl); patch coords
    h = ((as*2+al)*2+n2)*16+hl, w = (bs*2+bl)*32+ww."""
    a = pout.reshape(2, 2, 10, 2, 2, 3, 2, 16, 32)  # as bs k al bl c n2 hl ww
    a = a.transpose(2, 5, 0, 3, 6, 7, 1, 4, 8)      # k c as al n2 hl bs bl ww
    return np.ascontiguousarray(a.reshape(10, 3, 128, 128))


def _get_built():
    if "nc" not in _CACHE:
        _CACHE["nc"] = _build_nc()
        _CACHE["consts"] = _host_consts()
    return _CACHE["nc"], _CACHE["consts"]


def kernel(x_high: np.ndarray, scores_2d: np.ndarray):
    import jax
    import jax.numpy as jnp

    nc, (tri, ident, ssel) = _get_built()
    with jax.default_device(jax.devices("cpu")[0]):
        noise = np.asarray(
            jax.random.normal(jax.random.key(1), (N_CORES, NS, D), jnp.float32)
        )
    x_high = np.ascontiguousarray(x_high, dtype=np.float32)
    scores_2d = np.ascontiguousarray(scores_2d, dtype=np.float32)

    in_maps = [
        {
            "xblk": _blockize(x_high[b]),
            "scores": scores_2d[b].reshape(1, 256),
            "noise": np.ascontiguousarray(noise[b]),
            "tri": tri,
            "ident": ident,
            "ssel": ssel,
        }
        for b in range(N_CORES)
    ]
    res = run_bass_kernel_spmd(nc, in_maps, list(range(N_CORES)), **_RUN_KWARGS)
    _CACHE["last_res"] = res
    patches = np.stack([_unpack_out(res.results[b]["pout"]) for b in range(N_CORES)])
    entr = np.float32(np.mean([res.results[b]["entr"][0, 0] for b in range(N_CORES)]))
    return patches, entr
